# revision 49
# baseline (speedup 1.0000x reference)
"""Trainium2 Bass kernel for nn_CacheAttention (16-head causal MHA, T=2048 B=4 E=1024).

Sharding v2: core = (batch, head-half).  8 cores = 4 batches x 2 halves; each
core owns ONE batch and 8 heads (512 projection columns, processed as 4
partition-groups of 128 rows = 2 heads each).  vs v1 (2 heads x 4 batches per
core) this cuts per-core HBM traffic from 64 MB to 20 MB: q/k/v activations
for the core's batch are loaded once per rep and reused by all 4 groups.

Perf structure: the attention inner loop is ACT(exp)-bound, and the PE queue
is in-order, so every non-attention matmul (projections of the NEXT
partition-group, out-projection once the last group's chunk is normalized) is
emitted as fine-grained "feed" thunks BETWEEN attention instructions.  This
keeps the PE stream continuously busy, which both hides the work and holds
the PE at its top p-state (TRN2 DVFS: 2.4 GHz only after 3 us of
uninterrupted execution).  PV matmuls run one jp behind their exp (softmax
probabilities) so the PE never waits on ACT.  Score tiles are [128,1024]
PSUM (two s-tiles per exp); softmax sums ride along as a ones-column in V;
the causal path skips fully-masked tiles and uses 4 cached diagonal patterns.

Host sums each batch's two partial outputs and adds the output bias.
"""

import sys

if "/opt/trn_rl_repo" not in sys.path:
    sys.path.insert(0, "/opt/trn_rl_repo")

from collections import deque

import numpy as np
import ml_dtypes

import concourse.mybir as mybir
import concourse.tile as tile
from concourse import bacc
from concourse.bass_utils import run_bass_kernel_spmd
from concourse.masks import make_identity

BF16 = ml_dtypes.bfloat16
F32 = mybir.dt.float32
BF = mybir.dt.bfloat16

T, B, E = 2048, 4, 1024
H, D = 16, 64
NCORES = 8
HPC = 2                    # heads per partition-group
DC = 512                   # head-dim columns per core (8 heads)
NPG = DC // 128            # partition groups per core = 4
KT = E // 128              # E contraction tiles = 8
NCH = T // 512             # q chunks per (b,h) pair = 4
NST = T // 128             # s tiles = 16
SCALE = float(D) ** -0.5

_CACHE = {}


def _build(causal: bool, reps: int = 1, variant: str = "base"):
    # variant: base | dmaonly | noattn (attention replaced by memset) |
    #          noproj (projections replaced by memset) |
    #          noexp (exp+mask skipped; PV consumes stale pT) |
    #          nonorm (softmax normalization chain skipped)
    nc = bacc.Bacc("TRN2", target_bir_lowering=False, debug=False, num_devices=NCORES)

    xq_d = nc.dram_tensor("xq", [E, T], BF, kind="ExternalInput")
    xk_d = nc.dram_tensor("xk", [E, T], BF, kind="ExternalInput")
    xv_d = nc.dram_tensor("xv", [E, T], BF, kind="ExternalInput")
    wqT_d = nc.dram_tensor("wqT", [E, DC], BF, kind="ExternalInput")
    wkT_d = nc.dram_tensor("wkT", [E, DC], BF, kind="ExternalInput")
    wvT_d = nc.dram_tensor("wvT", [E, DC], BF, kind="ExternalInput")
    woT_d = nc.dram_tensor("woT", [DC, E], BF, kind="ExternalInput")
    bq_d = nc.dram_tensor("bq", [128, NPG], F32, kind="ExternalInput")
    bk_d = nc.dram_tensor("bk", [128, NPG], F32, kind="ExternalInput")
    bv_d = nc.dram_tensor("bv", [128, NPG], F32, kind="ExternalInput")
    if causal:
        dm_d = nc.dram_tensor("dmask", [4, 128, 512], BF, kind="ExternalInput")
    else:
        em_d = nc.dram_tensor("emaskT", [T, T], BF, kind="ExternalInput")
    out_d = nc.dram_tensor("out", [T, E], BF, kind="ExternalOutput")

    Exp = mybir.ActivationFunctionType.Exp
    add = mybir.AluOpType.add
    mult = mybir.AluOpType.mult
    big = variant == "bigact"

    with tile.TileContext(nc) as tc:
        with (
            tc.tile_pool(name="wp", bufs=1) as wp,
            tc.tile_pool(name="mp", bufs=1) as mp,
            tc.tile_pool(name="ps", bufs=2, space="PSUM") as ps,
        ):
            # ---- constants / weights (persistent) ----
            wq_sb = wp.tile([128, KT, DC], BF, tag="wq")
            wk_sb = wp.tile([128, KT, DC], BF, tag="wk")
            wv_sb = wp.tile([128, KT, DC], BF, tag="wv")
            for w_sb, w_d in ((wq_sb, wqT_d), (wk_sb, wkT_d), (wv_sb, wvT_d)):
                nc.sync.dma_start(w_sb, w_d.ap().rearrange("(k p) d -> p k d", p=128))
            wo_sb = wp.tile([128, NPG, E], BF, tag="wo")
            nc.sync.dma_start(wo_sb, woT_d.ap().rearrange("(g p) e -> p g e", p=128))
            bq_sb = wp.tile([128, NPG], F32, tag="bq")
            nc.sync.dma_start(bq_sb, bq_d.ap())
            bk_sb = wp.tile([128, NPG], F32, tag="bk")
            nc.sync.dma_start(bk_sb, bk_d.ap())
            bv_sb = wp.tile([128, NPG], F32, tag="bv")
            nc.sync.dma_start(bv_sb, bv_d.ap())
            ident = wp.tile([128, 128], BF, tag="ident")
            make_identity(nc, ident)
            # ones row at partition 64: stationary for the denominator
            # broadcast matmul, base-partition-matched to a_sb[64:65].
            ones_t = wp.tile([128, 64], BF, tag="ones_t")
            nc.vector.memset(ones_t, 1.0)
            # touch all pT rotation buffers once: the diag path skips exp on
            # fully-masked columns and relies on stale-but-finite contents
            # getting zeroed by the mask multiply.
            pt_shape, pt_bufs = ([128, 1024], 4) if big else ([128, 512], 8)
            for _ in range(pt_bufs):
                pT0 = mp.tile(pt_shape, BF, tag="pT", bufs=pt_bufs, name="pT")
                nc.vector.memset(pT0, 0.01)
            pdum = None
            if variant == "noexp":
                pdum = wp.tile([128, 512], BF, tag="pdum")
                nc.vector.memset(pdum, 0.01)
            if causal:
                dm_sb = wp.tile([128, 4 * 512], BF, tag="dm")
                nc.sync.dma_start(
                    dm_sb.rearrange("p (j q) -> p j q", q=512),
                    dm_d.ap().rearrange("j p q -> p j q"),
                )

            # rep-persistent activation tiles
            qT_all = wp.tile([128, NPG, T], BF, tag="qT")
            kT_all = wp.tile([128, NPG, T], BF, tag="kT")
            v_nat = wp.tile([128, NPG, NST * 130], BF, tag="vnat")
            attnT = wp.tile([128, NPG, T], BF, tag="attnT")

            def proj_thunks(pg):
                """Feed thunks computing qT/kT/v_nat for group pg: 3 x
                (dma + 2 psum-pair passes over 8 k-tiles + bias) + v
                transposes.  Each thunk is a small, stall-free PE burst."""
                thunks = []
                plan = (
                    (xq_d, wq_sb, bq_sb, SCALE, qT_all),
                    (xk_d, wk_sb, bk_sb, 1.0, kT_all),
                    (xv_d, wv_sb, bv_sb, 1.0, None),
                )
                xins = {}
                vTt = None

                def dma_xin(i):
                    def th():
                        src_d = plan[i][0]
                        xt = mp.tile([128, KT, T], BF, tag="xin", bufs=2, name="xin")
                        src_v = src_d.ap().rearrange("(k p) r -> p k r", p=128)
                        kk = KT // 2
                        for half in range(2):
                            nc.sync.dma_start(
                                xt[:, half * kk : (half + 1) * kk, :],
                                src_v[:, half * kk : (half + 1) * kk, :],
                            )
                        xins[i] = xt
                    return th

                def mk_group(i, g):
                    # one psum pair accumulating two 512-col chunks over all k
                    st = {}

                    def alloc():
                        st["pps"] = [
                            ps.tile([128, 512], F32, tag="mm", name=f"pp{j}")
                            for j in range(2)
                        ]

                    def mm(k):
                        def th():
                            if "pps" not in st:
                                alloc()
                            w_sb = plan[i][1]
                            for j in range(2):
                                n = 2 * g + j
                                nc.tensor.matmul(
                                    st["pps"][j],
                                    w_sb[:, k, 128 * pg : 128 * (pg + 1)],
                                    xins[i][:, k, 512 * n : 512 * (n + 1)],
                                    start=(k == 0),
                                    stop=(k == KT - 1),
                                )
                        return th

                    def bias():
                        nonlocal vTt
                        _, _, b_sb, scale, dst = plan[i]
                        if dst is None and vTt is None:
                            vTt = mp.tile([128, T], BF, tag="vTt", bufs=1)
                        for j in range(2):
                            n = 2 * g + j
                            dst_ap = (
                                vTt[:, 512 * n : 512 * (n + 1)]
                                if dst is None
                                else dst[:, pg, 512 * n : 512 * (n + 1)]
                            )
                            nc.vector.tensor_scalar(
                                dst_ap,
                                st["pps"][j],
                                b_sb[:, pg : pg + 1],
                                scale,
                                add,
                                mult,
                            )
                    return [mm(k) for k in range(KT)] + [bias]

                thunks.append(dma_xin(0))
                thunks.append(dma_xin(1))
                thunks.extend(mk_group(0, 0))
                thunks.extend(mk_group(0, 1))
                thunks.extend(mk_group(1, 0))
                thunks.append(dma_xin(2))
                thunks.extend(mk_group(1, 1))
                thunks.extend(mk_group(2, 0))
                thunks.extend(mk_group(2, 1))

                def ones():
                    vv = v_nat[:, pg, :].rearrange("p (r c) -> p r c", c=65)
                    nc.vector.memset(vv[:, :, 64], 1.0)
                thunks.append(ones)

                def transp(j):
                    def th():
                        pt = ps.tile([128, 128], BF, tag="mm", name="pt")
                        nc.tensor.transpose(pt, vTt[:, 128 * j : 128 * (j + 1)], ident)
                        for h in range(HPC):
                            nc.vector.tensor_copy(
                                v_nat[:, pg, 130 * j + 65 * h : 130 * j + 65 * h + 64],
                                pt[:, 64 * h : 64 * h + 64],
                            )
                    return th

                thunks.extend(transp(j) for j in range(NST))
                return thunks

            def outproj_chunk_thunks(c, st):
                # out rows 512c..512c+511; contraction over all 4 groups
                thunks = []

                def alloc():
                    st["o_big"] = mp.tile([128, 4, E], BF, tag="osb", bufs=2, name="o_big")

                def mk(r4, n):
                    def th():
                        if "o_big" not in st:
                            alloc()
                        r = 4 * c + r4
                        o_ps = ps.tile([128, 512], F32, tag="mm", name="o_ps")
                        for pg in range(NPG):
                            nc.tensor.matmul(
                                o_ps,
                                attnT[:, pg, 128 * r : 128 * (r + 1)],
                                wo_sb[:, pg, 512 * n : 512 * (n + 1)],
                                start=(pg == 0),
                                stop=(pg == NPG - 1),
                            )
                        nc.vector.tensor_copy(o_big_slice(st, r4, n), o_ps)
                    return th

                def o_big_slice(st, r4, n):
                    return st["o_big"][:, r4, 512 * n : 512 * (n + 1)]

                for r4 in range(4):
                    for n in range(E // 512):
                        thunks.append(mk(r4, n))

                def store():
                    nc.gpsimd.dma_start(
                        out_d.ap()[512 * c : 512 * (c + 1), :].rearrange(
                            "(r p) e -> p r e", p=128
                        ),
                        st.pop("o_big"),
                    )
                thunks.append(store)
                return thunks

            def attention(pg, feeds, on_chunk_done=None):
                """Attention for group pg (2 heads), feeding `feeds` thunks
                into the PE stream.  PV runs one jp behind its exp; the
                softmax normalization of chunk c-1 is emitted inside chunk
                c's first iteration (so its broadcast matmul never heads the
                PE queue while data is in flight).
                on_chunk_done(c) may extend `feeds` (out-proj release)."""
                def feed(k):
                    for _ in range(k):
                        if feeds:
                            feeds.popleft()()

                def drain(a_ps):
                    return a_ps

                def norm(c, a_ps):
                    # Runs one chunk late with double-buffered a_ps, off the
                    # critical path.  recip reads the PSUM ones-row directly;
                    # the broadcast is a K=1 PE matmul (GpSimd's
                    # partition_broadcast costs ~6us of Q7 time and paces the
                    # whole kernel if used here).
                    if variant == "nonorm":
                        if on_chunk_done is not None:
                            on_chunk_done(c)
                        return
                    for h in range(HPC):
                        hs = 64 * h
                        rl = mp.tile([1, 512], BF, tag="rl", bufs=4, name="rl")
                        nc.vector.tensor_copy(rl, a_ps[h][64:65, :])
                        if variant == "normprobe1":
                            continue
                        den_b = ps.tile([64, 512], F32, tag="mm", name="den_b")
                        nc.tensor.matmul(
                            den_b, ones_t[0:1, :], rl, start=True, stop=True
                        )
                        # InstReciprocal is column-serial (~2.6us for 512
                        # elems); the approx-fast custom op is one normal-rate
                        # DVE pass at ~18 correct bits, plenty above bf16.
                        rlb = mp.tile([64, 512], F32, tag="rlb", bufs=4, name="rlb")
                        nc.vector.reciprocal_approx_fast(rlb, den_b)
                        if variant == "normprobe2":
                            continue
                        nc.vector.tensor_tensor(
                            attnT[hs : hs + 64, pg, 512 * c : 512 * (c + 1)],
                            a_ps[h][0:64, :],
                            rlb,
                            mult,
                        )
                    if on_chunk_done is not None:
                        on_chunk_done(c)

                pending_norm = None

                for c in range(NCH):
                    n_s = 4 * (c + 1) if causal else NST
                    # bufs=2: chunk c+1 accumulates into the other buffer, so
                    # the (lagged) normalization of chunk c has a full chunk
                    # of slack before anything waits on its a_ps reads.
                    a_ps = [
                        ps.tile([65, 512], F32, tag=f"at{h}", bufs=2, name=f"a_ps{h}")
                        for h in range(HPC)
                    ]
                    prev = None

                    def emit_pv(item, n_s=n_s, a_ps=a_ps, pg=pg, c=c):
                        # pTs keyed (h, dj); each [128, 512].  Diagonal tiles
                        # contribute nothing to their fully-masked leading
                        # query columns, so slice them off (the chunk's first
                        # PV is never diagonal and initializes full width).
                        pTs, jpp = item
                        for h in range(HPC):
                            for dj in range(2):
                                j = 2 * jpp + dj
                                off = 128 * (j - 4 * c) if causal and j >= 4 * c else 0
                                start = jpp == 0 and dj == 0
                                if start:
                                    off = 0
                                nc.tensor.matmul(
                                    a_ps[h][:, off:],
                                    v_nat[
                                        :,
                                        pg,
                                        130 * j + 65 * h : 130 * j + 65 * (h + 1),
                                    ],
                                    pTs[(h, dj)][:, off:],
                                    start=start,
                                    stop=(jpp == n_s // 2 - 1 and dj == 1),
                                )

                    for jp in range(n_s // 2):
                        j0 = 2 * jp
                        if big:
                            # per-head [128,1024] score tile, bufs=2: h0/h1
                            # always land in the same buffer, so next jp's
                            # h0 QKs only wait for this jp's h0 exp (which
                            # ends one ACT earlier), hiding the PE<->ACT
                            # semaphore round-trip.  QK is h-grouped (no
                            # cross-head packing -- packing would couple the
                            # pair to BOTH buffers being free).
                            sch = [
                                ps.tile([128, 1024], F32, tag="sc", bufs=1, name="sc")
                                for _ in range(HPC)
                            ]
                            for h in range(HPC):
                                hs = 64 * h
                                for dj in range(2):
                                    j = j0 + dj
                                    nc.tensor.matmul(
                                        sch[h][:, 512 * dj : 512 * (dj + 1)],
                                        kT_all[hs : hs + 64, pg, 128 * j : 128 * (j + 1)],
                                        qT_all[hs : hs + 64, pg, 512 * c : 512 * (c + 1)],
                                        start=True,
                                        stop=True,
                                    )
                        else:
                            # one sc tag, bufs=4, allocated in the SAME
                            # (dj-major) order the ACTs free them; QK pairs
                            # (h0,h1)@dj are issued adjacently and pack into
                            # disjoint PE row-groups.
                            sc = {}
                            for dj in range(2):
                                for h in range(HPC):
                                    sc[(h, dj)] = ps.tile(
                                        [128, 512], F32, tag="sc", bufs=2, name="sc"
                                    )
                            qk_order = (
                                [(dj, h) for dj in range(2) for h in range(HPC)]
                                if variant != "nopack"
                                else [(dj, h) for h in range(HPC) for dj in range(2)]
                            )
                            for dj, h in qk_order:
                                j = j0 + dj
                                hs = 64 * h
                                off = (
                                    128 * (j - 4 * c)
                                    if causal and j >= 4 * c
                                    else 0
                                )
                                nc.tensor.matmul(
                                    sc[(h, dj)][:, off:],
                                    kT_all[hs : hs + 64, pg, 128 * j : 128 * (j + 1)],
                                    qT_all[
                                        hs : hs + 64,
                                        pg,
                                        512 * c + off : 512 * (c + 1),
                                    ],
                                    start=True,
                                    stop=True,
                                )
                        if pending_norm is not None:
                            norm(*pending_norm)
                            pending_norm = None
                        feed(1)
                        em0 = None
                        diag = False
                        if not causal:
                            em0 = mp.tile([128, 1024], BF, tag="em", bufs=4, name="em")
                            for dj in range(2):
                                nc.sync.dma_start(
                                    em0[:, 512 * dj : 512 * (dj + 1)],
                                    em_d.ap()[
                                        128 * (j0 + dj) : 128 * (j0 + dj + 1),
                                        512 * c : 512 * (c + 1),
                                    ],
                                )
                        elif j0 >= 4 * c:
                            diag = True
                            em0 = dm_sb[:, 512 * (j0 - 4 * c) : 512 * (j0 - 4 * c) + 1024]
                        pTs = {}
                        if big:
                            for h in range(HPC):
                                pT = mp.tile([128, 1024], BF, tag="pT", bufs=4, name="pT")
                                if variant != "noexp":
                                    nc.scalar.activation(pT, sch[h], Exp)
                                    if em0 is not None:
                                        for dj in range(2):
                                            dj4 = j0 + dj - 4 * c
                                            mw = (
                                                min(128 * (dj4 + 1), 512)
                                                if diag
                                                else 512
                                            )
                                            nc.vector.tensor_tensor(
                                                pT[:, 512 * dj : 512 * dj + mw],
                                                pT[:, 512 * dj : 512 * dj + mw],
                                                em0[:, 512 * dj : 512 * dj + mw],
                                                mult,
                                            )
                                for dj in range(2):
                                    pTs[(h, dj)] = pT[:, 512 * dj : 512 * (dj + 1)]
                        else:
                            for dj in range(2):
                                dj4 = j0 + dj - 4 * c  # diag pattern index
                                for h in range(HPC):
                                    if variant == "noexp":
                                        pTs[(h, dj)] = pdum
                                        continue
                                    pT = mp.tile(
                                        [128, 512], BF, tag="pT", bufs=8, name="pT"
                                    )
                                    if diag:
                                        # cols < 128*dj4 are fully masked:
                                        # skip the exp there (mask-mult
                                        # zeroes whatever the buffer held).
                                        off = 128 * dj4
                                        nc.scalar.activation(
                                            pT[:, off:], sc[(h, dj)][:, off:], Exp
                                        )
                                        mw = min(128 * (dj4 + 1), 512)
                                        nc.vector.tensor_tensor(
                                            pT[:, :mw],
                                            pT[:, :mw],
                                            em0[:, 512 * dj : 512 * dj + mw],
                                            mult,
                                        )
                                    else:
                                        nc.scalar.activation(pT, sc[(h, dj)], Exp)
                                        if em0 is not None:
                                            nc.vector.tensor_tensor(
                                                pT,
                                                pT,
                                                em0[:, 512 * dj : 512 * (dj + 1)],
                                                mult,
                                            )
                                    pTs[(h, dj)] = pT
                        if prev is not None:
                            emit_pv(prev)
                        feed(2)
                        prev = (pTs, jp)
                    if prev is not None:
                        emit_pv(prev)
                    pending_norm = (c, drain(a_ps) if variant != "nonorm" else None)
                if pending_norm is not None:
                    norm(*pending_norm)
                    pending_norm = None

            def memset_group(pg):
                nc.vector.memset(qT_all[:, pg, :], 0.02)
                nc.vector.memset(kT_all[:, pg, :], 0.02)
                nc.vector.memset(v_nat[:, pg, :], 0.01)

            # ================= main schedule =================
            proj0_fed = False
            for rep in range(reps):
                if variant in ("dmaonly", "noproj"):
                    # still move the input bytes
                    for src_d in (xq_d, xk_d, xv_d):
                        xt = mp.tile([128, KT, T], BF, tag="xin", bufs=2, name="xin")
                        nc.sync.dma_start(
                            xt, src_d.ap().rearrange("(k p) r -> p k r", p=128)
                        )
                    for pg in range(NPG):
                        memset_group(pg)
                elif not proj0_fed:
                    for th in proj_thunks(0):
                        th()

                if variant in ("dmaonly", "noattn"):
                    for pg in range(NPG):
                        nc.vector.memset(attnT[:, pg, :], 0.01)
                    if variant == "noattn":
                        for pg in range(1, NPG):
                            for th in proj_thunks(pg):
                                th()
                        st = {}
                        for c in range(NCH):
                            for th in outproj_chunk_thunks(c, st):
                                th()
                    else:
                        for c in range(NCH):
                            o_big = mp.tile(
                                [128, 4, E], BF, tag="osb", bufs=2, name="o_big"
                            )
                            nc.vector.memset(o_big, 0.0)
                            nc.gpsimd.dma_start(
                                out_d.ap()[512 * c : 512 * (c + 1), :].rearrange(
                                    "(r p) e -> p r e", p=128
                                ),
                                o_big,
                            )
                else:
                    if variant in ("nonorm", "normprobe1", "normprobe2"):
                        for pg in range(NPG):
                            nc.vector.memset(attnT[:, pg, :], 0.01)
                    for pg in range(NPG):
                        if pg < NPG - 1 and variant != "noproj":
                            feeds = deque(proj_thunks(pg + 1))
                        else:
                            feeds = deque()
                        if pg == NPG - 1:
                            # next rep's pg0 projections ride in this pg's
                            # feed stream too (removes the rep-start bubble)
                            proj0_fed = rep + 1 < reps and variant != "noproj"
                            if proj0_fed:
                                feeds.extend(proj_thunks(0))
                            op_state = {}

                            def release(c, feeds=feeds, op_state=op_state):
                                feeds.extend(outproj_chunk_thunks(c, op_state))

                            attention(pg, feeds, on_chunk_done=release)
                        else:
                            attention(pg, feeds)
                        while feeds:
                            feeds.popleft()()

    nc.compile()
    return nc


def _causal_mask_ref():
    return np.where(
        np.arange(T)[:, None] >= np.arange(T)[None, :], np.float32(0.0), np.float32(-1e9)
    ).astype(np.float32)


def _diag_patterns():
    # pattern[j, s, q] = 1.0 if (128*j + s) <= q else 0.0   (q in 0..511)
    j = np.arange(4)[:, None, None]
    s = np.arange(128)[None, :, None]
    q = np.arange(512)[None, None, :]
    return ((128 * j + s) <= q).astype(BF16)


def _prep_in_maps(query, key, value, attn_mask, wq, bq, wk, bk, wv, bv, wo, causal):
    common = {}
    if causal:
        common["dmask"] = np.ascontiguousarray(_diag_patterns())
    else:
        common["emaskT"] = np.exp(attn_mask.astype(np.float64).T).astype(BF16)
    in_maps = []
    xT = {}
    for b in range(B):
        xT[b] = {
            "xq": np.ascontiguousarray(query[:, b, :].T).astype(BF16),
            "xk": np.ascontiguousarray(key[:, b, :].T).astype(BF16),
            "xv": np.ascontiguousarray(value[:, b, :].T).astype(BF16),
        }
    for c in range(NCORES):
        b, hh = divmod(c, 2)
        sl = slice(DC * hh, DC * (hh + 1))
        m = dict(common)
        m.update(xT[b])
        m["wqT"] = np.ascontiguousarray(wq[sl, :].T).astype(BF16)
        m["wkT"] = np.ascontiguousarray(wk[sl, :].T).astype(BF16)
        m["wvT"] = np.ascontiguousarray(wv[sl, :].T).astype(BF16)
        m["woT"] = np.ascontiguousarray(wo[:, sl].T).astype(BF16)
        m["bq"] = np.ascontiguousarray(bq[sl].astype(np.float32).reshape(NPG, 128).T)
        m["bk"] = np.ascontiguousarray(bk[sl].astype(np.float32).reshape(NPG, 128).T)
        m["bv"] = np.ascontiguousarray(bv[sl].astype(np.float32).reshape(NPG, 128).T)
        in_maps.append(m)
    return in_maps


def _postprocess(results, bo):
    outs = []
    for b in range(B):
        acc = results[2 * b]["out"].astype(np.float32) + results[2 * b + 1][
            "out"
        ].astype(np.float32)
        outs.append(acc)
    out = np.stack(outs, axis=1) + bo[None, None, :]
    return np.ascontiguousarray(out.astype(np.float32))


def kernel(query, key, value, attn_mask, wq, bq, wk, bk, wv, bv, wo, bo):
    assert query.shape == (T, B, E), query.shape
    causal = bool(np.array_equal(attn_mask, _causal_mask_ref()))
    if causal not in _CACHE:
        _CACHE[causal] = _build(causal)
    nc = _CACHE[causal]
    in_maps = _prep_in_maps(
        query, key, value, attn_mask, wq, bq, wk, bk, wv, bv, wo, causal
    )
    res = run_bass_kernel_spmd(nc, in_maps, core_ids=list(range(NCORES)))
    return _postprocess(res.results, np.asarray(bo, dtype=np.float32))


# revision 53
# speedup vs baseline: 1.1279x; 1.1279x over previous
"""Trainium2 Bass kernel for nn_CacheAttention (16-head causal MHA, T=2048 B=4 E=1024).

Sharding v2: core = (batch, head-half).  8 cores = 4 batches x 2 halves; each
core owns ONE batch and 8 heads (512 projection columns, processed as 4
partition-groups of 128 rows = 2 heads each).  vs v1 (2 heads x 4 batches per
core) this cuts per-core HBM traffic from 64 MB to 20 MB: q/k/v activations
for the core's batch are loaded once per rep and reused by all 4 groups.

Perf structure: the attention inner loop is ACT(exp)-bound, and the PE queue
is in-order, so every non-attention matmul (projections of the NEXT
partition-group, out-projection once the last group's chunk is normalized) is
emitted as fine-grained "feed" thunks BETWEEN attention instructions.  This
keeps the PE stream continuously busy, which both hides the work and holds
the PE at its top p-state (TRN2 DVFS: 2.4 GHz only after 3 us of
uninterrupted execution).  PV matmuls run one jp behind their exp (softmax
probabilities) so the PE never waits on ACT.  Score tiles are [128,1024]
PSUM (two s-tiles per exp); softmax sums ride along as a ones-column in V;
the causal path skips fully-masked tiles and uses 4 cached diagonal patterns.

Host sums each batch's two partial outputs and adds the output bias.
"""

import sys

if "/opt/trn_rl_repo" not in sys.path:
    sys.path.insert(0, "/opt/trn_rl_repo")

from collections import deque

import numpy as np
import ml_dtypes

import concourse.mybir as mybir
import concourse.tile as tile
from concourse import bacc
from concourse.bass_utils import run_bass_kernel_spmd
from concourse.masks import make_identity

BF16 = ml_dtypes.bfloat16
F32 = mybir.dt.float32
BF = mybir.dt.bfloat16

T, B, E = 2048, 4, 1024
H, D = 16, 64
NCORES = 8
HPC = 2                    # heads per partition-group
DC = 512                   # head-dim columns per core (8 heads)
NPG = DC // 128            # partition groups per core = 4
KT = E // 128              # E contraction tiles = 8
NCH = T // 512             # q chunks per (b,h) pair = 4
NST = T // 128             # s tiles = 16
SCALE = float(D) ** -0.5

_CACHE = {}


def _build(causal: bool, reps: int = 1, variant: str = "base"):
    # variant: base | dmaonly | noattn (attention replaced by memset) |
    #          noproj (projections replaced by memset) |
    #          noexp (exp+mask skipped; PV consumes stale pT) |
    #          nonorm (softmax normalization chain skipped)
    nc = bacc.Bacc("TRN2", target_bir_lowering=False, debug=False, num_devices=NCORES)

    xq_d = nc.dram_tensor("xq", [E, T], BF, kind="ExternalInput")
    xk_d = nc.dram_tensor("xk", [E, T], BF, kind="ExternalInput")
    xv_d = nc.dram_tensor("xv", [E, T], BF, kind="ExternalInput")
    wqT_d = nc.dram_tensor("wqT", [E, DC], BF, kind="ExternalInput")
    wkT_d = nc.dram_tensor("wkT", [E, DC], BF, kind="ExternalInput")
    wvT_d = nc.dram_tensor("wvT", [E, DC], BF, kind="ExternalInput")
    woT_d = nc.dram_tensor("woT", [DC, E], BF, kind="ExternalInput")
    bq_d = nc.dram_tensor("bq", [128, NPG], F32, kind="ExternalInput")
    bk_d = nc.dram_tensor("bk", [128, NPG], F32, kind="ExternalInput")
    bv_d = nc.dram_tensor("bv", [128, NPG], F32, kind="ExternalInput")
    if causal:
        dm_d = nc.dram_tensor("dmask", [4, 128, 512], BF, kind="ExternalInput")
    else:
        em_d = nc.dram_tensor("emaskT", [T, T], BF, kind="ExternalInput")
    out_d = nc.dram_tensor("out", [T, E], BF, kind="ExternalOutput")

    Exp = mybir.ActivationFunctionType.Exp
    add = mybir.AluOpType.add
    mult = mybir.AluOpType.mult
    big = variant == "bigact"
    dskip = variant != "noskip"

    with tile.TileContext(nc) as tc:
        with (
            tc.tile_pool(name="wp", bufs=1) as wp,
            tc.tile_pool(name="mp", bufs=1) as mp,
            tc.tile_pool(name="ps", bufs=2, space="PSUM") as ps,
        ):
            # ---- constants / weights (persistent) ----
            wq_sb = wp.tile([128, KT, DC], BF, tag="wq")
            wk_sb = wp.tile([128, KT, DC], BF, tag="wk")
            wv_sb = wp.tile([128, KT, DC], BF, tag="wv")
            for w_sb, w_d in ((wq_sb, wqT_d), (wk_sb, wkT_d), (wv_sb, wvT_d)):
                nc.sync.dma_start(w_sb, w_d.ap().rearrange("(k p) d -> p k d", p=128))
            wo_sb = wp.tile([128, NPG, E], BF, tag="wo")
            nc.sync.dma_start(wo_sb, woT_d.ap().rearrange("(g p) e -> p g e", p=128))
            bq_sb = wp.tile([128, NPG], F32, tag="bq")
            nc.sync.dma_start(bq_sb, bq_d.ap())
            bk_sb = wp.tile([128, NPG], F32, tag="bk")
            nc.sync.dma_start(bk_sb, bk_d.ap())
            bv_sb = wp.tile([128, NPG], F32, tag="bv")
            nc.sync.dma_start(bv_sb, bv_d.ap())
            ident = wp.tile([128, 128], BF, tag="ident")
            make_identity(nc, ident)
            # ones row at partition 64: stationary for the denominator
            # broadcast matmul, base-partition-matched to a_sb[64:65].
            ones_t = wp.tile([128, 64], BF, tag="ones_t")
            nc.vector.memset(ones_t, 1.0)
            # touch all pT rotation buffers once: the diag path skips exp on
            # fully-masked columns and relies on stale-but-finite contents
            # getting zeroed by the mask multiply.
            pt_shape, pt_bufs = ([128, 1024], 4) if big else ([128, 512], 8)
            for _ in range(pt_bufs):
                pT0 = mp.tile(pt_shape, BF, tag="pT", bufs=pt_bufs, name="pT")
                nc.vector.memset(pT0, 0.01)
            pdum = None
            if variant == "noexp":
                pdum = wp.tile([128, 512], BF, tag="pdum")
                nc.vector.memset(pdum, 0.01)
            if causal:
                dm_sb = wp.tile([128, 4 * 512], BF, tag="dm")
                nc.sync.dma_start(
                    dm_sb.rearrange("p (j q) -> p j q", q=512),
                    dm_d.ap().rearrange("j p q -> p j q"),
                )

            # rep-persistent activation tiles
            qT_all = wp.tile([128, NPG, T], BF, tag="qT")
            kT_all = wp.tile([128, NPG, T], BF, tag="kT")
            v_nat = wp.tile([128, NPG, NST * 130], BF, tag="vnat")
            attnT = wp.tile([128, NPG, T], BF, tag="attnT")

            def proj_thunks(pg):
                """Feed thunks computing qT/kT/v_nat for group pg: 3 x
                (dma + 2 psum-pair passes over 8 k-tiles + bias) + v
                transposes.  Each thunk is a small, stall-free PE burst."""
                thunks = []
                plan = (
                    (xq_d, wq_sb, bq_sb, SCALE, qT_all),
                    (xk_d, wk_sb, bk_sb, 1.0, kT_all),
                    (xv_d, wv_sb, bv_sb, 1.0, None),
                )
                xins = {}
                vTt = None

                def dma_xin(i):
                    def th():
                        src_d = plan[i][0]
                        xt = mp.tile([128, KT, T], BF, tag="xin", bufs=2, name="xin")
                        src_v = src_d.ap().rearrange("(k p) r -> p k r", p=128)
                        kk = KT // 2
                        for half in range(2):
                            nc.sync.dma_start(
                                xt[:, half * kk : (half + 1) * kk, :],
                                src_v[:, half * kk : (half + 1) * kk, :],
                            )
                        xins[i] = xt
                    return th

                def mk_group(i, g):
                    # one psum pair accumulating two 512-col chunks over all k
                    st = {}

                    def alloc():
                        st["pps"] = [
                            ps.tile([128, 512], F32, tag="mm", name=f"pp{j}")
                            for j in range(2)
                        ]

                    def mm(k):
                        def th():
                            if "pps" not in st:
                                alloc()
                            w_sb = plan[i][1]
                            for j in range(2):
                                n = 2 * g + j
                                nc.tensor.matmul(
                                    st["pps"][j],
                                    w_sb[:, k, 128 * pg : 128 * (pg + 1)],
                                    xins[i][:, k, 512 * n : 512 * (n + 1)],
                                    start=(k == 0),
                                    stop=(k == KT - 1),
                                )
                        return th

                    def bias():
                        nonlocal vTt
                        _, _, b_sb, scale, dst = plan[i]
                        if dst is None and vTt is None:
                            vTt = mp.tile([128, T], BF, tag="vTt", bufs=1)
                        for j in range(2):
                            n = 2 * g + j
                            dst_ap = (
                                vTt[:, 512 * n : 512 * (n + 1)]
                                if dst is None
                                else dst[:, pg, 512 * n : 512 * (n + 1)]
                            )
                            nc.vector.tensor_scalar(
                                dst_ap,
                                st["pps"][j],
                                b_sb[:, pg : pg + 1],
                                scale,
                                add,
                                mult,
                            )
                    return [mm(k) for k in range(KT)] + [bias]

                thunks.append(dma_xin(0))
                thunks.append(dma_xin(1))
                thunks.extend(mk_group(0, 0))
                thunks.extend(mk_group(0, 1))
                thunks.extend(mk_group(1, 0))
                thunks.append(dma_xin(2))
                thunks.extend(mk_group(1, 1))
                thunks.extend(mk_group(2, 0))
                thunks.extend(mk_group(2, 1))

                def ones():
                    vv = v_nat[:, pg, :].rearrange("p (r c) -> p r c", c=65)
                    nc.vector.memset(vv[:, :, 64], 1.0)
                thunks.append(ones)

                def transp(j):
                    def th():
                        pt = ps.tile([128, 128], BF, tag="mm", name="pt")
                        nc.tensor.transpose(pt, vTt[:, 128 * j : 128 * (j + 1)], ident)
                        for h in range(HPC):
                            nc.vector.tensor_copy(
                                v_nat[:, pg, 130 * j + 65 * h : 130 * j + 65 * h + 64],
                                pt[:, 64 * h : 64 * h + 64],
                            )
                    return th

                thunks.extend(transp(j) for j in range(NST))
                return thunks

            def outproj_chunk_thunks(c, st):
                # out rows 512c..512c+511; contraction over all 4 groups
                thunks = []

                def alloc():
                    st["o_big"] = mp.tile([128, 4, E], BF, tag="osb", bufs=2, name="o_big")

                def mk(r4, n):
                    def th():
                        if "o_big" not in st:
                            alloc()
                        r = 4 * c + r4
                        o_ps = ps.tile([128, 512], F32, tag="mm", name="o_ps")
                        for pg in range(NPG):
                            nc.tensor.matmul(
                                o_ps,
                                attnT[:, pg, 128 * r : 128 * (r + 1)],
                                wo_sb[:, pg, 512 * n : 512 * (n + 1)],
                                start=(pg == 0),
                                stop=(pg == NPG - 1),
                            )
                        nc.vector.tensor_copy(o_big_slice(st, r4, n), o_ps)
                    return th

                def o_big_slice(st, r4, n):
                    return st["o_big"][:, r4, 512 * n : 512 * (n + 1)]

                for r4 in range(4):
                    for n in range(E // 512):
                        thunks.append(mk(r4, n))

                def store():
                    nc.gpsimd.dma_start(
                        out_d.ap()[512 * c : 512 * (c + 1), :].rearrange(
                            "(r p) e -> p r e", p=128
                        ),
                        st.pop("o_big"),
                    )
                thunks.append(store)
                return thunks

            def attention(pg, feeds, on_chunk_done=None):
                """Attention for group pg (2 heads), feeding `feeds` thunks
                into the PE stream.  PV runs one jp behind its exp; the
                softmax normalization of chunk c-1 is emitted inside chunk
                c's first iteration (so its broadcast matmul never heads the
                PE queue while data is in flight).
                on_chunk_done(c) may extend `feeds` (out-proj release)."""
                def feed(k):
                    for _ in range(k):
                        if feeds:
                            feeds.popleft()()

                def drain(a_ps):
                    return a_ps

                def norm(c, a_ps):
                    # Runs one chunk late with double-buffered a_ps, off the
                    # critical path.  recip reads the PSUM ones-row directly;
                    # the broadcast is a K=1 PE matmul (GpSimd's
                    # partition_broadcast costs ~6us of Q7 time and paces the
                    # whole kernel if used here).
                    if variant == "nonorm":
                        if on_chunk_done is not None:
                            on_chunk_done(c)
                        return
                    for h in range(HPC):
                        hs = 64 * h
                        rl = mp.tile([1, 512], BF, tag="rl", bufs=4, name="rl")
                        nc.vector.tensor_copy(rl, a_ps[h][64:65, :])
                        if variant == "normprobe1":
                            continue
                        den_b = ps.tile([64, 512], F32, tag="mm", name="den_b")
                        nc.tensor.matmul(
                            den_b, ones_t[0:1, :], rl, start=True, stop=True
                        )
                        # InstReciprocal is column-serial (~2.6us for 512
                        # elems); the approx-fast custom op is one normal-rate
                        # DVE pass at ~18 correct bits, plenty above bf16.
                        rlb = mp.tile([64, 512], F32, tag="rlb", bufs=4, name="rlb")
                        nc.vector.reciprocal_approx_fast(rlb, den_b)
                        if variant == "normprobe2":
                            continue
                        nc.vector.tensor_tensor(
                            attnT[hs : hs + 64, pg, 512 * c : 512 * (c + 1)],
                            a_ps[h][0:64, :],
                            rlb,
                            mult,
                        )
                    if on_chunk_done is not None:
                        on_chunk_done(c)

                pending_norm = None

                for c in range(NCH):
                    n_s = 4 * (c + 1) if causal else NST
                    # bufs=2: chunk c+1 accumulates into the other buffer, so
                    # the (lagged) normalization of chunk c has a full chunk
                    # of slack before anything waits on its a_ps reads.
                    a_ps = [
                        ps.tile([65, 512], F32, tag=f"at{h}", bufs=2, name=f"a_ps{h}")
                        for h in range(HPC)
                    ]
                    prev = None

                    def emit_pv(item, n_s=n_s, a_ps=a_ps, pg=pg, c=c):
                        # pTs keyed (h, dj); each [128, 512].  Diagonal tiles
                        # contribute nothing to their fully-masked leading
                        # query columns, so slice them off (the chunk's first
                        # PV is never diagonal and initializes full width).
                        pTs, jpp = item
                        for h in range(HPC):
                            for dj in range(2):
                                j = 2 * jpp + dj
                                off = (
                                    128 * (j - 4 * c)
                                    if dskip and causal and j >= 4 * c
                                    else 0
                                )
                                start = jpp == 0 and dj == 0
                                if start:
                                    off = 0
                                nc.tensor.matmul(
                                    a_ps[h][:, off:],
                                    v_nat[
                                        :,
                                        pg,
                                        130 * j + 65 * h : 130 * j + 65 * (h + 1),
                                    ],
                                    pTs[(h, dj)][:, off:],
                                    start=start,
                                    stop=(jpp == n_s // 2 - 1 and dj == 1),
                                )

                    for jp in range(n_s // 2):
                        j0 = 2 * jp
                        if big:
                            # per-head [128,1024] score tile, bufs=2: h0/h1
                            # always land in the same buffer, so next jp's
                            # h0 QKs only wait for this jp's h0 exp (which
                            # ends one ACT earlier), hiding the PE<->ACT
                            # semaphore round-trip.  QK is h-grouped (no
                            # cross-head packing -- packing would couple the
                            # pair to BOTH buffers being free).
                            sch = [
                                ps.tile([128, 1024], F32, tag="sc", bufs=1, name="sc")
                                for _ in range(HPC)
                            ]
                            for h in range(HPC):
                                hs = 64 * h
                                for dj in range(2):
                                    j = j0 + dj
                                    nc.tensor.matmul(
                                        sch[h][:, 512 * dj : 512 * (dj + 1)],
                                        kT_all[hs : hs + 64, pg, 128 * j : 128 * (j + 1)],
                                        qT_all[hs : hs + 64, pg, 512 * c : 512 * (c + 1)],
                                        start=True,
                                        stop=True,
                                    )
                        else:
                            # one sc tag, bufs=4, allocated in the SAME
                            # (dj-major) order the ACTs free them; QK pairs
                            # (h0,h1)@dj are issued adjacently and pack into
                            # disjoint PE row-groups.
                            sc = {}
                            for dj in range(2):
                                for h in range(HPC):
                                    sc[(h, dj)] = ps.tile(
                                        [128, 512], F32, tag="sc", bufs=2, name="sc"
                                    )
                            qk_order = (
                                [(dj, h) for dj in range(2) for h in range(HPC)]
                                if variant != "nopack"
                                else [(dj, h) for h in range(HPC) for dj in range(2)]
                            )
                            for dj, h in qk_order:
                                j = j0 + dj
                                hs = 64 * h
                                off = (
                                    128 * (j - 4 * c)
                                    if dskip and causal and j >= 4 * c
                                    else 0
                                )
                                nc.tensor.matmul(
                                    sc[(h, dj)][:, off:],
                                    kT_all[hs : hs + 64, pg, 128 * j : 128 * (j + 1)],
                                    qT_all[
                                        hs : hs + 64,
                                        pg,
                                        512 * c + off : 512 * (c + 1),
                                    ],
                                    start=True,
                                    stop=True,
                                )
                        if pending_norm is not None:
                            norm(*pending_norm)
                            pending_norm = None
                        feed(1)
                        em0 = None
                        diag = False
                        if not causal:
                            em0 = mp.tile([128, 1024], BF, tag="em", bufs=3, name="em")
                            for dj in range(2):
                                nc.sync.dma_start(
                                    em0[:, 512 * dj : 512 * (dj + 1)],
                                    em_d.ap()[
                                        128 * (j0 + dj) : 128 * (j0 + dj + 1),
                                        512 * c : 512 * (c + 1),
                                    ],
                                )
                        elif j0 >= 4 * c:
                            diag = True
                            em0 = dm_sb[:, 512 * (j0 - 4 * c) : 512 * (j0 - 4 * c) + 1024]
                        pTs = {}
                        if big:
                            for h in range(HPC):
                                pT = mp.tile([128, 1024], BF, tag="pT", bufs=4, name="pT")
                                if variant != "noexp":
                                    nc.scalar.activation(pT, sch[h], Exp)
                                    if em0 is not None:
                                        for dj in range(2):
                                            dj4 = j0 + dj - 4 * c
                                            mw = (
                                                min(128 * (dj4 + 1), 512)
                                                if diag
                                                else 512
                                            )
                                            nc.vector.tensor_tensor(
                                                pT[:, 512 * dj : 512 * dj + mw],
                                                pT[:, 512 * dj : 512 * dj + mw],
                                                em0[:, 512 * dj : 512 * dj + mw],
                                                mult,
                                            )
                                for dj in range(2):
                                    pTs[(h, dj)] = pT[:, 512 * dj : 512 * (dj + 1)]
                        else:
                            for dj in range(2):
                                dj4 = j0 + dj - 4 * c  # diag pattern index
                                for h in range(HPC):
                                    if variant == "noexp":
                                        pTs[(h, dj)] = pdum
                                        continue
                                    pT = mp.tile(
                                        [128, 512], BF, tag="pT", bufs=8, name="pT"
                                    )
                                    if diag:
                                        # cols < 128*dj4 are fully masked:
                                        # skip the exp there (mask-mult
                                        # zeroes whatever the buffer held).
                                        off = 128 * dj4
                                        nc.scalar.activation(
                                            pT[:, off:], sc[(h, dj)][:, off:], Exp
                                        )
                                        mw = min(128 * (dj4 + 1), 512)
                                        nc.vector.tensor_tensor(
                                            pT[:, :mw],
                                            pT[:, :mw],
                                            em0[:, 512 * dj : 512 * dj + mw],
                                            mult,
                                        )
                                    else:
                                        nc.scalar.activation(pT, sc[(h, dj)], Exp)
                                        if em0 is not None:
                                            nc.vector.tensor_tensor(
                                                pT,
                                                pT,
                                                em0[:, 512 * dj : 512 * (dj + 1)],
                                                mult,
                                            )
                                    pTs[(h, dj)] = pT
                        if prev is not None:
                            emit_pv(prev)
                        feed(2)
                        prev = (pTs, jp)
                    if prev is not None:
                        emit_pv(prev)
                    pending_norm = (c, drain(a_ps) if variant != "nonorm" else None)
                if pending_norm is not None:
                    norm(*pending_norm)
                    pending_norm = None

            def memset_group(pg):
                nc.vector.memset(qT_all[:, pg, :], 0.02)
                nc.vector.memset(kT_all[:, pg, :], 0.02)
                nc.vector.memset(v_nat[:, pg, :], 0.01)

            # ================= main schedule =================
            proj0_fed = False
            for rep in range(reps):
                if variant in ("dmaonly", "noproj"):
                    # still move the input bytes
                    for src_d in (xq_d, xk_d, xv_d):
                        xt = mp.tile([128, KT, T], BF, tag="xin", bufs=2, name="xin")
                        nc.sync.dma_start(
                            xt, src_d.ap().rearrange("(k p) r -> p k r", p=128)
                        )
                    for pg in range(NPG):
                        memset_group(pg)
                elif not proj0_fed:
                    for th in proj_thunks(0):
                        th()

                if variant in ("dmaonly", "noattn"):
                    for pg in range(NPG):
                        nc.vector.memset(attnT[:, pg, :], 0.01)
                    if variant == "noattn":
                        for pg in range(1, NPG):
                            for th in proj_thunks(pg):
                                th()
                        st = {}
                        for c in range(NCH):
                            for th in outproj_chunk_thunks(c, st):
                                th()
                    else:
                        for c in range(NCH):
                            o_big = mp.tile(
                                [128, 4, E], BF, tag="osb", bufs=2, name="o_big"
                            )
                            nc.vector.memset(o_big, 0.0)
                            nc.gpsimd.dma_start(
                                out_d.ap()[512 * c : 512 * (c + 1), :].rearrange(
                                    "(r p) e -> p r e", p=128
                                ),
                                o_big,
                            )
                else:
                    if variant in ("nonorm", "normprobe1", "normprobe2"):
                        for pg in range(NPG):
                            nc.vector.memset(attnT[:, pg, :], 0.01)
                    for pg in range(NPG):
                        if pg < NPG - 1 and variant != "noproj":
                            feeds = deque(proj_thunks(pg + 1))
                        else:
                            feeds = deque()
                        if pg == NPG - 1:
                            # next rep's pg0 projections ride in this pg's
                            # feed stream too (removes the rep-start bubble)
                            proj0_fed = rep + 1 < reps and variant != "noproj"
                            if proj0_fed:
                                feeds.extend(proj_thunks(0))
                            op_state = {}

                            def release(c, feeds=feeds, op_state=op_state):
                                feeds.extend(outproj_chunk_thunks(c, op_state))

                            attention(pg, feeds, on_chunk_done=release)
                        else:
                            attention(pg, feeds)
                        while feeds:
                            feeds.popleft()()

    nc.compile()
    return nc


def _causal_mask_ref():
    return np.where(
        np.arange(T)[:, None] >= np.arange(T)[None, :], np.float32(0.0), np.float32(-1e9)
    ).astype(np.float32)


def _diag_patterns():
    # pattern[j, s, q] = 1.0 if (128*j + s) <= q else 0.0   (q in 0..511)
    j = np.arange(4)[:, None, None]
    s = np.arange(128)[None, :, None]
    q = np.arange(512)[None, None, :]
    return ((128 * j + s) <= q).astype(BF16)


def _prep_in_maps(query, key, value, attn_mask, wq, bq, wk, bk, wv, bv, wo, causal):
    common = {}
    if causal:
        common["dmask"] = np.ascontiguousarray(_diag_patterns())
    else:
        common["emaskT"] = np.exp(attn_mask.astype(np.float64).T).astype(BF16)
    in_maps = []
    xT = {}
    for b in range(B):
        xT[b] = {
            "xq": np.ascontiguousarray(query[:, b, :].T).astype(BF16),
            "xk": np.ascontiguousarray(key[:, b, :].T).astype(BF16),
            "xv": np.ascontiguousarray(value[:, b, :].T).astype(BF16),
        }
    for c in range(NCORES):
        b, hh = divmod(c, 2)
        sl = slice(DC * hh, DC * (hh + 1))
        m = dict(common)
        m.update(xT[b])
        m["wqT"] = np.ascontiguousarray(wq[sl, :].T).astype(BF16)
        m["wkT"] = np.ascontiguousarray(wk[sl, :].T).astype(BF16)
        m["wvT"] = np.ascontiguousarray(wv[sl, :].T).astype(BF16)
        m["woT"] = np.ascontiguousarray(wo[:, sl].T).astype(BF16)
        m["bq"] = np.ascontiguousarray(bq[sl].astype(np.float32).reshape(NPG, 128).T)
        m["bk"] = np.ascontiguousarray(bk[sl].astype(np.float32).reshape(NPG, 128).T)
        m["bv"] = np.ascontiguousarray(bv[sl].astype(np.float32).reshape(NPG, 128).T)
        in_maps.append(m)
    return in_maps


def _postprocess(results, bo):
    outs = []
    for b in range(B):
        acc = results[2 * b]["out"].astype(np.float32) + results[2 * b + 1][
            "out"
        ].astype(np.float32)
        outs.append(acc)
    out = np.stack(outs, axis=1) + bo[None, None, :]
    return np.ascontiguousarray(out.astype(np.float32))


def kernel(query, key, value, attn_mask, wq, bq, wk, bk, wv, bv, wo, bo):
    assert query.shape == (T, B, E), query.shape
    causal = bool(np.array_equal(attn_mask, _causal_mask_ref()))
    if causal not in _CACHE:
        _CACHE[causal] = _build(causal)
    nc = _CACHE[causal]
    in_maps = _prep_in_maps(
        query, key, value, attn_mask, wq, bq, wk, bk, wv, bv, wo, causal
    )
    res = run_bass_kernel_spmd(nc, in_maps, core_ids=list(range(NCORES)))
    return _postprocess(res.results, np.asarray(bo, dtype=np.float32))
